# revision 5
# baseline (speedup 1.0000x reference)
"""Trainium2 Bass kernel for nn_CustomProjectionModel (scatter_memory).

Computation: flat = P @ u  (P: [2099712, 64], u: [64, 1]) scattered into a
2-layer MLP's params (W1 [2048,512], b1, W2 [512,2048], b2), then
out = relu(x @ W1.T + b1) @ W2.T + b2  for x [256, 512].

Strategy (8 NeuronCores):
  - Shard the GEMV (the memory-bound part, ~537 MB of P) row-wise: core k owns
    W1's hidden slice j in [256k, 256k+256), the matching b1 slice, W2's
    *column* slice (same hidden j range), and a replicated copy of b2's rows.
  - Host pre-arranges each core's P rows into tiles so that:
      * TensorE path: P rows are pre-transposed into matmul moving operands;
        a block-diagonal-u stationary computes 128 dot products per 2
        output rows, accumulating into PSUM so results land *directly* in
        the final lhsT layouts needed by the MLP (no on-device transposes).
      * VectorE path: remaining rows packed [128, 64, 64]; fp32
        multiply + free-axis reduce.
    Splitting between PE and DVE keeps both engines below the HBM roofline.
  - MLP runs tensor-parallel (hidden sharded); partial outputs are
    AllReduce'd on-device; every core writes the same [512, 256] out^T.
"""

import sys

if "/opt/trn_rl_repo" not in sys.path:
    sys.path.insert(0, "/opt/trn_rl_repo")

import numpy as np

IN_DIM, HID_DIM, OUT_DIM, M_RANK = 512, 2048, 512, 64
N_W1 = HID_DIM * IN_DIM            # 1048576
N_B1 = HID_DIM                     # 2048
N_W2 = OUT_DIM * HID_DIM           # 1048576
N_B2 = OUT_DIM                     # 512
OFF_W1, OFF_B1 = 0, N_W1
OFF_W2, OFF_B2 = N_W1 + N_B1, N_W1 + N_B1 + N_W2
TOTAL = OFF_B2 + N_B2              # 2099712
BATCH = 256
N_CORES = 8

# PE path: 3 PSUM "sets" (banks) x 4 col-groups x 16 matmuls, each matmul
# K=128 (2 P-rows interleaved), N=512.
N_SETS = 3          # set 0,1 -> LT1 (W1^T tiles); set 2 -> LT2 half 0
N_MM = N_SETS * 64  # 192 matmuls
# DVE path: 8 tiles of [128, 64 rows, 64] for LT2 half 1, plus bias tile.
N_DVE = 8

_cache = {}


def _core_indices(k):
    """Flat-row index arrays for core k's host-side data layout."""
    jb = 256 * k
    p = np.arange(128, dtype=np.int64)
    f = np.arange(512, dtype=np.int64)
    # psum partition for (colgroup b, matmul i, interleave s)
    part = (
        32 * np.arange(4, dtype=np.int64)[:, None, None]
        + 2 * np.arange(16, dtype=np.int64)[None, :, None]
        + np.arange(2, dtype=np.int64)[None, None, :]
    )  # [4, 16, 2]
    # Per-set flat-row formulas as a function of (psum partition pp, free f):
    # set 0/1 -> LT1[pp, 512*set + f], layout free=(c in 4, jj in 256):
    #   r = (jb + jj)*512 + 128*c + pp
    # set 2  -> LT2h0[pp, f]: r = OFF_W2 + f*2048 + jb + pp
    r_set = np.empty((3, 512), dtype=np.int64)
    c01 = f // 256
    jj = f % 256
    r_set[0] = (jb + jj) * 512 + 128 * c01
    r_set[1] = (jb + jj) * 512 + 128 * (2 + c01)
    r_set[2] = OFF_W2 + f * 2048 + jb
    rows_pe = part[None, :, :, :, None] + r_set[:, None, None, None, :]
    rows_pe = rows_pe.reshape(N_MM, 2, 512)  # [matmul, s, f]

    # DVE: LT2h1[p, f]: r = OFF_W2 + f*2048 + jb + 128 + p ; f = 64n + t
    n_ = np.arange(N_DVE, dtype=np.int64)
    t_ = np.arange(64, dtype=np.int64)
    rows_dve = (
        OFF_W2
        + (64 * n_[:, None, None] + t_[None, None, :]) * 2048
        + jb
        + 128
        + p[None, :, None]
    )  # [8, 128, 64]

    # bias: slots 0,1 = b1 halves; 2..5 = b2 quarters (replicated on all cores)
    rows_bias = np.stack(
        [
            OFF_B1 + jb + p,
            OFF_B1 + jb + 128 + p,
            OFF_B2 + p,
            OFF_B2 + 128 + p,
            OFF_B2 + 256 + p,
            OFF_B2 + 384 + p,
        ],
        axis=1,
    )  # [128, 6]
    return rows_pe, rows_dve, rows_bias


def _get_indices():
    if "idx" not in _cache:
        _cache["idx"] = [_core_indices(k) for k in range(N_CORES)]
    return _cache["idx"]


def _prep_inputs(x, P, u):
    """Build per-core input maps (host-side shard + relayout)."""
    x = np.ascontiguousarray(x, dtype=np.float32)
    P = np.ascontiguousarray(P, dtype=np.float32)
    u = np.ascontiguousarray(u, dtype=np.float32).reshape(M_RANK)

    # Shared across cores
    # xt_in[p, 256*c + b] = x[b, 128*c + p]
    xt_in = np.ascontiguousarray(
        x.reshape(BATCH, 4, 128).transpose(2, 1, 0).reshape(128, 4 * BATCH)
    )
    # u_rep[p, 64*t + m] = u[m]
    u_rep = np.ascontiguousarray(np.tile(u[None, :], (128, 64)))
    # Block-diagonal stationary: B[64*s + m, i, 2*i + s] = u[m]
    B = np.zeros((128, 16, 32), dtype=np.float32)
    i_ = np.arange(16)
    for s in (0, 1):
        B[64 * s + np.arange(64)[:, None], i_[None, :], 2 * i_[None, :] + s] = u[
            :, None
        ]
    b_in = np.ascontiguousarray(B.reshape(128, 512))

    in_maps = []
    for k in range(N_CORES):
        rows_pe, rows_dve, rows_bias = _get_indices()[k]
        pe = P[rows_pe]  # [192, 2, 512, 64]
        pe = pe.transpose(0, 1, 3, 2).reshape(N_MM, 128, 512)
        # group 4 matmuls per 1 MB DMA tile
        pe48 = np.ascontiguousarray(
            pe.reshape(N_MM // 4, 4, 128, 512)
            .transpose(0, 2, 1, 3)
            .reshape(N_MM // 4, 128, 2048)
        )
        dve = np.ascontiguousarray(P[rows_dve].reshape(N_DVE, 128, 64 * 64))
        bias = np.ascontiguousarray(P[rows_bias].reshape(128, 6 * 64))
        in_maps.append(
            {
                "pe_in": pe48,
                "dve_in": dve,
                "bias_in": bias,
                "b_in": b_in,
                "u_rep": u_rep,
                "xt_in": xt_in,
            }
        )
    return in_maps


def _emulate(in_maps):
    """Numpy emulation of the device program (for host-side validation)."""
    outs = []
    partials = []
    for k in range(N_CORES):
        im = in_maps[k]
        Bm = im["b_in"].reshape(128, 16, 32)
        pe = im["pe_in"].reshape(N_MM // 4, 128, 4, 512).transpose(0, 2, 1, 3)
        pe = pe.reshape(N_MM, 128, 512)
        lt1 = np.zeros((128, 1024), np.float32)
        lt20 = np.zeros((128, 512), np.float32)
        lt21 = np.zeros((128, 512), np.float32)
        for st in range(N_SETS):
            psum = np.zeros((128, 512), np.float32)
            for b in range(4):
                for i in range(16):
                    mi = st * 64 + b * 16 + i
                    # out[32b:32b+32] += B_i.T @ rhs
                    psum[32 * b : 32 * b + 32] += Bm[:, i, :].T @ pe[mi]
            if st < 2:
                lt1[:, 512 * st : 512 * st + 512] = psum
            else:
                lt20[:] = psum
        u_rep = im["u_rep"]
        for n in range(N_DVE):
            prod = im["dve_in"][n] * u_rep
            lt21[:, 64 * n : 64 * n + 64] = prod.reshape(128, 64, 64).sum(axis=2)
        prodb = im["bias_in"] * u_rep[:, : 6 * 64]
        bb = prodb.reshape(128, 6, 64).sum(axis=2)
        bb[:, 2:6] *= 0.125
        xt = im["xt_in"]
        hsb = np.zeros((128, 512), np.float32)
        for h in (0, 1):
            ps = np.zeros((128, 256), np.float32)
            for c in range(4):
                lhsT = lt1[:, 256 * c + 128 * h : 256 * c + 128 * h + 128]
                ps += lhsT.T @ xt[:, 256 * c : 256 * c + 256]
            hsb[:, 256 * h : 256 * h + 256] = np.maximum(ps + bb[:, h : h + 1], 0.0)
        part = np.zeros((512, 256), np.float32)
        for q in range(4):
            ps2 = np.zeros((128, 256), np.float32)
            for h in (0, 1):
                lt2 = lt20 if h == 0 else lt21
                lhsT = lt2[:, 128 * q : 128 * q + 128]
                ps2 += lhsT.T @ hsb[:, 256 * h : 256 * h + 256]
            part[128 * q : 128 * q + 128] = ps2 + bb[:, 2 + q : 3 + q]
        partials.append(part)
    ar = np.sum(partials, axis=0)
    for k in range(N_CORES):
        outs.append(ar)
    return outs


def _build_nc():
    """Build + compile the 8-core SPMD Bass program (cached)."""
    if "nc" in _cache:
        return _cache["nc"]

    from contextlib import ExitStack

    import concourse.bacc as bacc
    import concourse.tile as tile
    from concourse import mybir

    fp32 = mybir.dt.float32
    nc = bacc.Bacc(
        "TRN2",
        target_bir_lowering=False,
        debug=False,
        enable_asserts=False,
        num_devices=N_CORES,
    )

    pe_in = nc.dram_tensor("pe_in", [N_MM // 4, 128, 2048], fp32, kind="ExternalInput")
    dve_in = nc.dram_tensor("dve_in", [N_DVE, 128, 4096], fp32, kind="ExternalInput")
    bias_in = nc.dram_tensor("bias_in", [128, 384], fp32, kind="ExternalInput")
    b_in = nc.dram_tensor("b_in", [128, 512], fp32, kind="ExternalInput")
    u_rep_in = nc.dram_tensor("u_rep", [128, 4096], fp32, kind="ExternalInput")
    xt_in = nc.dram_tensor("xt_in", [128, 1024], fp32, kind="ExternalInput")
    out_ext = nc.dram_tensor("outT", [512, 256], fp32, kind="ExternalOutput")

    partial_d = nc.dram_tensor("partial_d", [512, 256], fp32)
    ar_out = nc.dram_tensor("ar_out", [512, 256], fp32)

    with tile.TileContext(nc) as tc, ExitStack() as ctx:
        consts = ctx.enter_context(tc.tile_pool(name="consts", bufs=1))
        res = ctx.enter_context(tc.tile_pool(name="res", bufs=1))
        pe_pool = ctx.enter_context(tc.tile_pool(name="pe_rhs", bufs=6))
        dve_pool = ctx.enter_context(tc.tile_pool(name="dve_t", bufs=3))
        prod_pool = ctx.enter_context(tc.tile_pool(name="prod", bufs=2))
        psum_pe = ctx.enter_context(tc.tile_pool(name="psum_pe", bufs=4, space="PSUM"))
        psum_mlp = ctx.enter_context(
            tc.tile_pool(name="psum_mlp", bufs=2, space="PSUM")
        )

        b_sb = consts.tile([128, 512], fp32)
        nc.sync.dma_start(b_sb[:], b_in[:, :])
        urep_sb = consts.tile([128, 4096], fp32)
        nc.sync.dma_start(urep_sb[:], u_rep_in[:, :])
        xt_sb = consts.tile([128, 1024], fp32)
        nc.sync.dma_start(xt_sb[:], xt_in[:, :])
        bias_sb = consts.tile([128, 384], fp32)
        nc.sync.dma_start(bias_sb[:], bias_in[:, :])

        lt1 = res.tile([128, 1024], fp32)     # W1^T: free = (c in 4, j in 256)
        lt20 = res.tile([128, 512], fp32)     # W2 cols, j half 0: free = o
        lt21 = res.tile([128, 512], fp32)     # W2 cols, j half 1: free = o
        bb = res.tile([128, 6], fp32)         # b1 halves + b2/8 quarters
        hsb = res.tile([128, 512], fp32)      # relu hidden, free = (h, batch)
        parts = res.tile([128, 1024], fp32)   # partial out^T, free = (q, batch)

        b_sb3 = b_sb[:].rearrange("p (i w) -> p i w", i=16)

        # ---- TensorE GEMV path ----
        for st in range(N_SETS):
            psum = psum_pe.tile([128, 512], fp32)
            for b in range(4):
                for i in range(16):
                    mi = st * 64 + b * 16 + i
                    g, jj = divmod(mi, 4)
                    if jj == 0:
                        rhs = pe_pool.tile([128, 2048], fp32)
                        nc.sync.dma_start(rhs[:], pe_in[g, :, :])
                    nc.tensor.matmul(
                        psum[32 * b : 32 * b + 32, :],
                        b_sb3[:, i, :],
                        rhs[:, 512 * jj : 512 * jj + 512],
                        start=(i == 0),
                        stop=(i == 15),
                        tile_position=(0, 32 * b),
                    )
            for b in range(4):
                if st < 2:
                    dst = lt1[32 * b : 32 * b + 32, 512 * st : 512 * st + 512]
                else:
                    dst = lt20[32 * b : 32 * b + 32, :]
                nc.vector.tensor_copy(dst, psum[32 * b : 32 * b + 32, :])

        # ---- VectorE GEMV path ----
        for n in range(N_DVE):
            t = dve_pool.tile([128, 4096], fp32)
            nc.sync.dma_start(t[:], dve_in[n, :, :])
            prod = prod_pool.tile([128, 4096], fp32)
            nc.vector.tensor_mul(prod[:], t[:], urep_sb[:])
            nc.vector.tensor_reduce(
                lt21[:, 64 * n : 64 * n + 64],
                prod[:].rearrange("p (t m) -> p t m", m=64),
                axis=mybir.AxisListType.X,
                op=mybir.AluOpType.add,
            )

        # bias GEMV (b1 local slice, b2 replicated -> scale 1/8)
        prodb = prod_pool.tile([128, 4096], fp32)
        nc.vector.tensor_mul(prodb[:, 0:384], bias_sb[:], urep_sb[:, 0:384])
        nc.vector.tensor_reduce(
            bb[:],
            prodb[:, 0:384].rearrange("p (t m) -> p t m", m=64),
            axis=mybir.AxisListType.X,
            op=mybir.AluOpType.add,
        )
        nc.vector.tensor_scalar_mul(bb[:, 2:6], bb[:, 2:6], 0.125)

        # ---- MLP layer 1: h^T[j, b] = relu(W1_k @ x^T + b1) ----
        for h in (0, 1):
            ps = psum_mlp.tile([128, 256], fp32, tag="mlp")
            for c in range(4):
                nc.tensor.matmul(
                    ps[:],
                    lt1[:, 256 * c + 128 * h : 256 * c + 128 * h + 128],
                    xt_sb[:, 256 * c : 256 * c + 256],
                    start=(c == 0),
                    stop=(c == 3),
                )
            nc.scalar.activation(
                hsb[:, 256 * h : 256 * h + 256],
                ps[:],
                mybir.ActivationFunctionType.Relu,
                bias=bb[:, h : h + 1],
                scale=1.0,
            )

        # ---- MLP layer 2 partials: out^T[o, b] = W2_k^T-slice @ h + b2/8 ----
        for q in range(4):
            ps2 = psum_mlp.tile([128, 256], fp32, tag="mlp")
            for h in (0, 1):
                lt2 = lt20 if h == 0 else lt21
                nc.tensor.matmul(
                    ps2[:],
                    lt2[:, 128 * q : 128 * q + 128],
                    hsb[:, 256 * h : 256 * h + 256],
                    start=(h == 0),
                    stop=(h == 1),
                )
            nc.scalar.activation(
                parts[:, 256 * q : 256 * q + 256],
                ps2[:],
                mybir.ActivationFunctionType.Identity,
                bias=bb[:, 2 + q : 3 + q],
                scale=1.0,
            )
            nc.sync.dma_start(
                partial_d[128 * q : 128 * q + 128, :],
                parts[:, 256 * q : 256 * q + 256],
            )

        # ---- cross-core AllReduce of partial outputs ----
        nc.gpsimd.collective_compute(
            "AllReduce",
            mybir.AluOpType.add,
            replica_groups=[list(range(N_CORES))],
            ins=[partial_d.ap()],
            outs=[ar_out.ap()],
        )
        nc.sync.dma_start(out_ext[:, :], ar_out[:, :])

    nc.compile()
    _cache["nc"] = nc
    return nc


KERNEL_TRACE = False  # set True (e.g. from test.py) to capture an NTFF profile


def kernel(x, P, u):
    in_maps = _prep_inputs(x, P, u)
    nc = _build_nc()

    from concourse.bass_utils import run_bass_kernel_spmd

    res = run_bass_kernel_spmd(
        nc, in_maps, core_ids=list(range(N_CORES)), trace=KERNEL_TRACE
    )
    _cache["last_results"] = res
    outT = res.results[0]["outT"]
    return np.ascontiguousarray(outT.T).astype(np.float32)
